# revision 18
# baseline (speedup 1.0000x reference)
"""ConvexMixer Trainium2 kernel.

Sharding: 8 cores; core c handles batch b=c//4, heads {2*(c%4), 2*(c%4)+1}.
Math reformulation (validated vs reference, rel-err ~1.4e-6):
  - q-side hull term fq is a per-row additive constant in the row softmax -> dropped.
  - 1e-5 jax noise terms dropped (forward delta is ~1e-6 relative).
  - exp(gk_j) folded into phi_k column scale: scores = tau_i * ln((phiq+eps)@((phik+eps)*e^gk)^T * mask).
  - gate 1-exp(-softplus(u)) == sigmoid(u).
  - no max-subtraction softmax: logits bounded in [-40, 27] (fp32-safe).
Outputs: out [B,H,S,D]; a [B,S] finished on host from per-core A partials
(A = sum_h normalized-w; summing partials across 4 cores of a batch is the
only cross-core reduction).
"""

import numpy as np
import ml_dtypes

B, H, S, D = 2, 8, 1024, 64
P, D1, R = 8, 128, 16
NCHUNK = S // 128          # 8 i-chunks of 128
EPS = 1e-6
TAU_A, TAU_B = 0.30343, 0.22159

_cache = {}


def _softplus(x):
    return np.log1p(np.exp(-np.abs(x))) + np.maximum(x, 0.0)


def _sigmoid(x):
    return 1.0 / (1.0 + np.exp(-x))


def _build_program():
    import concourse.bacc as bacc
    import concourse.bass as bass
    import concourse.tile as tile
    from concourse import mybir
    from concourse.hw_specs import get_activation_tables as _gat

    # All activations here are Exp/Ln. The greedy table-load pass alternates
    # between per-func tables, inserting ~85 ACT_TABLE_LOADs (~110us). Pin
    # every activation to the one set containing both exp and ln.
    def _gat_one(arch):
        t = _gat(arch)
        keep = 'natural_log_exp_and_others'
        assert keep in t
        return {n: (s if n == keep else set()) for n, s in t.items()}
    bacc.get_activation_tables = _gat_one

    f32 = mybir.dt.float32
    f32r = mybir.dt.float32r
    bf16 = mybir.dt.bfloat16
    AF = mybir.ActivationFunctionType
    OP = mybir.AluOpType
    AX = mybir.AxisListType

    nc = bacc.Bacc(None, target_bir_lowering=False, debug=False)

    # ---- DRAM I/O ----
    qh = nc.dram_tensor("qh", [2, NCHUNK, 128, D], f32, kind="ExternalInput")
    kh = nc.dram_tensor("kh", [2, NCHUNK, 128, D], f32, kind="ExternalInput")
    vh = nc.dram_tensor("vh", [2, NCHUNK, 128, D], f32, kind="ExternalInput")
    maskh = nc.dram_tensor("maskh", [2, NCHUNK, 128, S], f32, kind="ExternalInput")
    w0aug_d = nc.dram_tensor("w0aug", [D + 1, P * D1], f32r, kind="ExternalInput")
    w1eff_d = nc.dram_tensor("w1eff", [D1, P * D], f32r, kind="ExternalInput")
    b1g1_d = nc.dram_tensor("b1g1", [D, P], f32, kind="ExternalInput")
    onescol_d = nc.dram_tensor("onescol", [D, P * P], f32r, kind="ExternalInput")
    obm_d = nc.dram_tensor("obm", [1, P], f32r, kind="ExternalInput")
    gw_d = nc.dram_tensor("gw", [1, D], f32, kind="ExternalInput")
    gb_d = nc.dram_tensor("gb", [1, 1], f32, kind="ExternalInput")
    whkT_d = nc.dram_tensor("whkT", [D, R], f32r, kind="ExternalInput")
    whqT_d = nc.dram_tensor("whqT", [D, R], f32r, kind="ExternalInput")
    ident_d = nc.dram_tensor("ident", [128, 128], f32, kind="ExternalInput")
    ones_d = nc.dram_tensor("ones", [1, S], f32r, kind="ExternalInput")
    identb_d = nc.dram_tensor("identb", [128, 128], bf16, kind="ExternalInput")
    outh = nc.dram_tensor("outh", [2, NCHUNK, 128, D], f32, kind="ExternalOutput")
    Ap_d = nc.dram_tensor("Ap", [NCHUNK, 128, S], f32, kind="ExternalOutput")

    def r32(ap):
        return ap.bitcast(f32r)

    with tile.TileContext(nc) as tc:
        with (
            tc.tile_pool(name="consts", bufs=1) as consts,
            tc.tile_pool(name="perbh", bufs=2) as perbh,
            tc.tile_pool(name="amem", bufs=1) as amem,
            tc.tile_pool(name="work", bufs=2) as work,
            tc.tile_pool(name="mwork", bufs=3) as mwork,
            tc.tile_pool(name="small", bufs=4) as small,
            tc.tile_pool(name="ps_big", bufs=3, space="PSUM") as ps_big,
            tc.tile_pool(name="ps_med", bufs=2, space="PSUM") as ps_med,
            tc.tile_pool(name="dram", bufs=2, space="DRAM") as dpool,
        ):
            # ---------- constants ----------
            w0aug = consts.tile([D + 1, P * D1], f32r)
            nc.sync.dma_start(out=w0aug, in_=w0aug_d[:])
            w1eff = consts.tile([D1, P * D], f32r)
            nc.sync.dma_start(out=w1eff, in_=w1eff_d[:])
            b1g1 = consts.tile([D, P], f32)
            nc.sync.dma_start(out=b1g1, in_=b1g1_d[:])
            onescol = consts.tile([D, P * P], f32r)
            nc.sync.dma_start(out=onescol, in_=onescol_d[:])
            obm = consts.tile([1, P], f32r)
            nc.sync.dma_start(out=obm, in_=obm_d[:])
            whkT = consts.tile([D, R], f32r)
            nc.sync.dma_start(out=whkT, in_=whkT_d[:])
            whqT = consts.tile([D, R], f32r)
            nc.sync.dma_start(out=whqT, in_=whqT_d[:])
            ident = consts.tile([128, 128], f32)
            nc.sync.dma_start(out=ident, in_=ident_d[:])
            identb = consts.tile([128, 128], bf16)
            nc.sync.dma_start(out=identb, in_=identb_d[:])
            gwb = consts.tile([128, D], f32)
            nc.sync.dma_start(
                out=gwb,
                in_=bass.AP(tensor=gw_d[:].tensor, offset=gw_d[:].offset,
                            ap=[[0, 128]] + gw_d[:].ap[-1:]),
            )
            gbb = consts.tile([128, 1], f32)
            nc.sync.dma_start(
                out=gbb,
                in_=bass.AP(tensor=gb_d[:].tensor, offset=gb_d[:].offset,
                            ap=[[0, 128]] + gb_d[:].ap[-1:]),
            )
            onesrow = consts.tile([1, S], f32r)
            nc.sync.dma_start(out=onesrow, in_=ones_d[:])
            tbias = consts.tile([128, 1], f32)
            nc.vector.memset(tbias, TAU_B)

            # ---------- per-bh persistent state ----------
            st = [{} for _ in range(2)]

            # ============ PHASE 1: sigmoid gate ============
            for bh in range(2):
                d = st[bh]
                k_all = perbh.tile([128, NCHUNK, D], f32, tag="k_all")
                nc.sync.dma_start(out=k_all, in_=kh[bh].rearrange("c p d -> p c d"))
                d["k_all"] = k_all
                ut = work.tile([128, NCHUNK, D], f32, tag="ut")
                for c in range(NCHUNK):
                    nc.vector.tensor_mul(ut[:, c, :], k_all[:, c, :], gwb)
                u = small.tile([128, NCHUNK], f32, tag="u")
                nc.vector.reduce_sum(u, ut, axis=AX.X)
                nc.vector.tensor_scalar(u, u, gbb, None, op0=OP.add)
                g_all = perbh.tile([128, NCHUNK], f32, tag="g_all")
                nc.scalar.activation(g_all, u, AF.Exp, scale=-1.0)
                nc.vector.tensor_scalar(g_all, g_all, 1.0, None, op0=OP.add)
                nc.vector.reciprocal(g_all, g_all)
                d["g_all"] = g_all

            # ============ PHASE 2: softplus (hull, phi, q-gate) ============
            for bh in range(2):
                d = st[bh]
                k_all, g_all = d["k_all"], d["g_all"]
                # xg = k * g (per-token scalar)
                xg_all = perbh.tile([128, NCHUNK, D], f32, tag="xg_all")
                for c in range(NCHUNK):
                    nc.vector.tensor_scalar(
                        xg_all[:, c, :], k_all[:, c, :], g_all[:, c:c + 1], None,
                        op0=OP.mult)
                # mk = mean(xg^2) + eps
                sq = work.tile([128, NCHUNK, D], f32, tag="sq")
                nc.vector.tensor_mul(sq, xg_all, xg_all)
                mk = perbh.tile([128, NCHUNK], f32, tag="mk")
                nc.vector.reduce_sum(mk, sq, axis=AX.X)
                nc.vector.tensor_scalar(mk, mk, 1.0 / D, EPS, op0=OP.mult, op1=OP.add)
                d["mk"] = mk

                # transposes: kT (raw k) and xgT, with ones row at 64
                kTaug = perbh.tile([D + 1, S], f32r, tag="kTaug")
                xgTaug = perbh.tile([D + 1, S], f32r, tag="xgTaug")
                nc.sync.dma_start(out=kTaug[D:D + 1, :], in_=ones_d[:])
                nc.sync.dma_start(out=xgTaug[D:D + 1, :], in_=ones_d[:])
                for c in range(NCHUNK):
                    tp = ps_med.tile([128, S // NCHUNK], f32, tag="medb")
                    nc.tensor.transpose(tp[0:D, 0:128], k_all[:, c, :], ident)
                    nc.vector.tensor_copy(kTaug[0:D, c * 128:(c + 1) * 128],
                                          tp[0:D, 0:128])
                    tp2 = ps_med.tile([128, S // NCHUNK], f32, tag="medb")
                    nc.tensor.transpose(tp2[0:D, 0:128], xg_all[:, c, :], ident)
                    nc.vector.tensor_copy(xgTaug[0:D, c * 128:(c + 1) * 128],
                                          tp2[0:D, 0:128])
                d["kTaug"] = kTaug

                # hull: z0 -> z1 -> petal means accumulated into s_ps
                s_ps = ps_big.tile([128, S], f32, tag="big")
                for p in range(P):
                    z0ps = ps_big.tile([128, S], f32, tag="big")
                    for hf in range(2):
                        nc.tensor.matmul(
                            z0ps[:, hf * 512:(hf + 1) * 512],
                            r32(w0aug[:, p * D1:(p + 1) * D1]),
                            r32(xgTaug[:, hf * 512:(hf + 1) * 512]),
                            start=True, stop=True)
                    z0e = work.tile([128, S], f32, tag="ze")
                    nc.scalar.activation(z0e, z0ps, AF.Exp)
                    z0sb = work.tile([128, S], f32r, tag="z0sb")
                    nc.scalar.activation(z0sb, z0e, AF.Ln, bias=1.0)
                    z1ps = ps_big.tile([128, S], f32, tag="big")
                    for hf in range(2):
                        nc.tensor.matmul(
                            z1ps[0:D, hf * 512:(hf + 1) * 512],
                            r32(w1eff[:, p * D:(p + 1) * D]),
                            r32(z0sb[:, hf * 512:(hf + 1) * 512]),
                            start=True, stop=True)
                    z1e = work.tile([D, S], f32, tag="ze")
                    nc.scalar.activation(z1e, z1ps[0:D, :], AF.Exp,
                                         bias=b1g1[:, p:p + 1])
                    z1sb = work.tile([D, S], f32r, tag="z1sb")
                    nc.scalar.activation(z1sb, z1e, AF.Ln, bias=1.0)
                    for hf in range(2):
                        nc.tensor.matmul(
                            s_ps[0:P, hf * 512:(hf + 1) * 512],
                            r32(onescol[:, p * P:(p + 1) * P]),
                            r32(z1sb[:, hf * 512:(hf + 1) * 512]),
                            start=(p == 0), stop=False)
                for hf in range(2):
                    nc.tensor.matmul(
                        s_ps[0:P, hf * 512:(hf + 1) * 512],
                        r32(obm), r32(onesrow[:, hf * 512:(hf + 1) * 512]),
                        start=False, stop=True)
                s_sb = perbh.tile([P, S], f32, tag="s_sb")
                nc.vector.tensor_copy(s_sb, s_ps[0:P, :])
                d["s_sb"] = s_sb

                # phi_k from raw kT
                phk_ps = ps_big.tile([128, S], f32, tag="big")
                for hf in range(2):
                    nc.tensor.matmul(
                        phk_ps[0:R, hf * 512:(hf + 1) * 512],
                        r32(whkT), r32(kTaug[0:D, hf * 512:(hf + 1) * 512]),
                        start=True, stop=True)
                phk = perbh.tile([R, S], f32, tag="phk")
                nc.vector.tensor_scalar(phk, phk_ps[0:R, :], 20.0, None, op0=OP.min)
                nc.scalar.activation(phk, phk, AF.Exp)
                nc.scalar.activation(phk, phk, AF.Ln, bias=1.0)
                nc.vector.tensor_scalar(phk, phk, EPS, None, op0=OP.add)
                d["phk"] = phk

                # q side
                q_all = perbh.tile([128, NCHUNK, D], f32, tag="q_all")
                nc.sync.dma_start(out=q_all, in_=qh[bh].rearrange("c p d -> p c d"))
                gq = work.tile([128, NCHUNK, D], f32, tag="gq")
                nc.scalar.activation(gq, q_all, AF.Exp)
                nc.scalar.activation(gq, gq, AF.Ln, bias=1.0)
                qp_all = perbh.tile([128, NCHUNK, D], f32, tag="qp_all")
                nc.vector.tensor_mul(qp_all, q_all, gq)
                sq2 = work.tile([128, NCHUNK, D], f32, tag="sq")
                nc.vector.tensor_mul(sq2, qp_all, qp_all)
                mq = perbh.tile([128, NCHUNK], f32, tag="mq")
                nc.vector.reduce_sum(mq, sq2, axis=AX.X)
                nc.vector.tensor_scalar(mq, mq, 1.0 / D, EPS, op0=OP.mult, op1=OP.add)
                d["mq"] = mq
                qpT = perbh.tile([D, S], f32r, tag="qpT")
                for c in range(NCHUNK):
                    tp = ps_med.tile([128, S // NCHUNK], f32, tag="medb")
                    nc.tensor.transpose(tp[0:D, 0:128], qp_all[:, c, :], ident)
                    nc.vector.tensor_copy(qpT[:, c * 128:(c + 1) * 128],
                                          tp[0:D, 0:128])
                phq_ps = ps_big.tile([128, S], f32, tag="big")
                for hf in range(2):
                    nc.tensor.matmul(
                        phq_ps[0:R, hf * 512:(hf + 1) * 512],
                        r32(whqT), r32(qpT[:, hf * 512:(hf + 1) * 512]),
                        start=True, stop=True)
                phq = perbh.tile([R, S], f32r, tag="phq")
                nc.vector.tensor_scalar(phq, phq_ps[0:R, :], 20.0, None, op0=OP.min)
                nc.scalar.activation(phq, phq, AF.Exp)
                nc.scalar.activation(phq, phq, AF.Ln, bias=1.0)
                nc.vector.tensor_scalar(phq, phq, EPS, None, op0=OP.add)
                d["phq"] = phq

                # v load + bf16 cast
                v_all = work.tile([128, NCHUNK, D], f32, tag="v_all")
                nc.sync.dma_start(out=v_all, in_=vh[bh].rearrange("c p d -> p c d"))
                v_bf = perbh.tile([128, NCHUNK, D], bf16, tag="v_bf")
                nc.vector.tensor_copy(v_bf, v_all)
                d["v_bf"] = v_bf

            # ============ r = sqrt(m) = exp(0.5*ln(m)) ============
            for bh in range(2):
                d = st[bh]
                rk = perbh.tile([128, NCHUNK], f32, tag="rk")
                nc.scalar.activation(rk, d["mk"], AF.Ln)
                nc.scalar.activation(rk, rk, AF.Exp, scale=0.5)
                rq = perbh.tile([128, NCHUNK], f32, tag="rq")
                nc.scalar.activation(rq, d["mq"], AF.Ln)
                nc.scalar.activation(rq, rq, AF.Exp, scale=0.5)
                d["rk"], d["rq"] = rk, rq

            # ============ PHASE 4: ln/exp ============
            for bh in range(2):
                d = st[bh]
                tauq = perbh.tile([128, NCHUNK], f32, tag="tauq")
                nc.scalar.activation(tauq, d["rq"], AF.Exp, bias=tbias, scale=TAU_A)
                d["tauq"] = tauq
                tauk = small.tile([128, NCHUNK], f32, tag="tauk")
                nc.scalar.activation(tauk, d["rk"], AF.Exp, bias=tbias, scale=TAU_A)
                itauk = small.tile([128, NCHUNK], f32, tag="itauk")
                nc.vector.reciprocal(itauk, tauk)
                # petal logsumexp -> egk
                egk = perbh.tile([128, NCHUNK], f32, tag="egk")
                s_sb = d["s_sb"]
                for c in range(NCHUNK):
                    stp = ps_med.tile([128, P], f32, tag="medb")
                    nc.tensor.transpose(stp[:, 0:P],
                                        s_sb[:, c * 128:(c + 1) * 128],
                                        ident[0:P, 0:P])
                    est = small.tile([128, P], f32, tag="est")
                    nc.vector.tensor_scalar(est, stp[0:128, 0:P],
                                            tauk[:, c:c + 1], None, op0=OP.mult)
                    zs = small.tile([128, 1], f32, tag="zs")
                    nc.scalar.activation(est, est, AF.Exp, accum_out=zs)
                    lzs = small.tile([128, 1], f32, tag="lzs")
                    nc.scalar.activation(lzs, zs, AF.Ln)
                    nc.vector.tensor_scalar(lzs, lzs, itauk[:, c:c + 1], None,
                                            op0=OP.mult)
                    nc.scalar.activation(egk[:, c:c + 1], lzs, AF.Exp)
                # egk -> DRAM -> broadcast row [R, S]
                scr = dpool.tile([1, S], f32)
                nc.sync.dma_start(
                    out=bass.AP(tensor=scr.tensor, offset=scr.offset,
                                ap=[[1, 128], [128, NCHUNK]]),
                    in_=egk)
                egkb = perbh.tile([R, S], f32, tag="egkb")
                nc.sync.dma_start(
                    out=egkb,
                    in_=bass.AP(tensor=scr.tensor, offset=scr.offset,
                                ap=[[0, R]] + scr.ap[-1:]))
                phkp = perbh.tile([R, S], f32r, tag="phkp")
                nc.vector.tensor_mul(phkp, d["phk"], egkb)
                d["phkp"] = phkp

            # scores + softmax + wv + A
            A_tiles = []
            for c in range(NCHUNK):
                A_c = amem.tile([128, S], f32, tag=f"A{c}", name=f"A{c}")
                A_tiles.append(A_c)
            for bh in range(2):
                d = st[bh]
                phq, phkp, tauq, v_bf = d["phq"], d["phkp"], d["tauq"], d["v_bf"]
                for c in range(NCHUNK):
                    kp_ps = ps_big.tile([128, S], f32, tag="big")
                    for hf in range(2):
                        nc.tensor.matmul(
                            kp_ps[:, hf * 512:(hf + 1) * 512],
                            r32(phq[:, c * 128:(c + 1) * 128]),
                            r32(phkp[:, hf * 512:(hf + 1) * 512]),
                            start=True, stop=True)
                    mt = mwork.tile([128, S], f32, tag="mt")
                    nc.sync.dma_start(out=mt, in_=maskh[bh, c])
                    msb = mwork.tile([128, S], f32, tag="msb")
                    nc.vector.tensor_mul(msb, kp_ps, mt)
                    nc.scalar.activation(msb, msb, AF.Ln)
                    wt_bf = mwork.tile([128, S], bf16, tag="wt_bf")
                    zrow = small.tile([128, 1], f32, tag="zrow")
                    nc.scalar.activation(wt_bf, msb, AF.Exp,
                                         scale=tauq[:, c:c + 1], accum_out=zrow)
                    rz = small.tile([128, 1], f32, tag="rz")
                    nc.vector.reciprocal(rz, zrow)
                    wtT_ps = ps_med.tile([128, S], bf16, tag="medb")
                    for jb in range(NCHUNK):
                        nc.tensor.transpose(
                            wtT_ps[:, jb * 128:(jb + 1) * 128],
                            wt_bf[:, jb * 128:(jb + 1) * 128], identb)
                    wtT = mwork.tile([128, S], bf16, tag="wtT")
                    nc.any.tensor_copy(wtT, wtT_ps)
                    ov_ps = ps_med.tile([128, D], f32, tag="medb")
                    for jb in range(NCHUNK):
                        nc.tensor.matmul(
                            ov_ps,
                            wtT[:, jb * 128:(jb + 1) * 128],
                            v_bf[:, jb, :],
                            start=(jb == 0), stop=(jb == NCHUNK - 1))
                    osb = small.tile([128, D], f32, tag="osb")
                    nc.vector.tensor_scalar(osb, ov_ps, rz, None,
                                            op0=OP.mult)
                    nc.sync.dma_start(out=outh[bh, c], in_=osb)
                    # A accumulation (normalized w)
                    if bh == 0:
                        nc.vector.tensor_scalar(A_tiles[c], wt_bf, rz, None,
                                                op0=OP.mult)
                    else:
                        wn = work.tile([128, S], f32, tag="ze")
                        nc.vector.tensor_scalar(wn, wt_bf, rz, None, op0=OP.mult)
                        nc.vector.tensor_add(A_tiles[c], A_tiles[c], wn)
                        nc.sync.dma_start(out=Ap_d[c], in_=A_tiles[c])

    nc.compile()
    return nc


def _host_prep(inputs):
    p = {n: np.asarray(inputs[f'k_{n}'], np.float64)
         for n in ['w0', 'b0', 'w1', 'b1', 'zw', 'g0', 'g1', 'ob', 'gw', 'gb']}
    w0 = _softplus(p['w0']) ** 2
    w1 = _softplus(p['w1']) ** 2
    g0 = _sigmoid(p['g0'])
    g1 = _sigmoid(p['g1'])
    w0aug = np.zeros((D + 1, P * D1), np.float32)
    for pe in range(P):
        w0aug[:D, pe * D1:(pe + 1) * D1] = (w0[pe].T * g0[pe]).astype(np.float32)
        w0aug[D, pe * D1:(pe + 1) * D1] = (p['b0'][pe] * g0[pe]).astype(np.float32)
    w1eff = np.zeros((D1, P * D), np.float32)
    b1g1 = np.zeros((D, P), np.float32)
    onescol = np.zeros((D, P * P), np.float32)
    for pe in range(P):
        w1eff[:, pe * D:(pe + 1) * D] = (w1[pe].T * g1[pe] + p['zw'][pe]).astype(np.float32)
        b1g1[:, pe] = (p['b1'][pe] * g1[pe]).astype(np.float32)
        onescol[:, pe * P + pe] = 1.0 / D
    obm = p['ob'].mean(-1).astype(np.float32).reshape(1, P)
    return {
        'w0aug': w0aug, 'w1eff': w1eff, 'b1g1': b1g1, 'onescol': onescol,
        'obm': obm,
        'gw': p['gw'].astype(np.float32).reshape(1, D),
        'gb': p['gb'].astype(np.float32).reshape(1, 1),
        'whkT': np.asarray(inputs['whk'], np.float32).T.copy(),
        'whqT': np.asarray(inputs['whq'], np.float32).T.copy(),
        'ident': np.eye(128, dtype=np.float32),
        'ones': np.ones((1, S), np.float32),
        'identb': np.eye(128).astype(ml_dtypes.bfloat16),
    }


def kernel(**inputs):
    from concourse.bass_utils import run_bass_kernel_spmd

    if 'nc' not in _cache:
        _cache['nc'] = _build_program()
    nc = _cache['nc']

    shared = _host_prep(inputs)
    q = np.ascontiguousarray(np.asarray(inputs['q'], np.float32))
    k = np.ascontiguousarray(np.asarray(inputs['k'], np.float32))
    v = np.ascontiguousarray(np.asarray(inputs['v'], np.float32))
    mask = np.ascontiguousarray(np.asarray(inputs['mask'], np.float32))

    in_maps = []
    for core in range(8):
        b = core // 4
        h0 = 2 * (core % 4)
        m = dict(shared)
        m['qh'] = q[b, h0:h0 + 2].reshape(2, NCHUNK, 128, D).copy()
        m['kh'] = k[b, h0:h0 + 2].reshape(2, NCHUNK, 128, D).copy()
        m['vh'] = v[b, h0:h0 + 2].reshape(2, NCHUNK, 128, D).copy()
        m['maskh'] = mask[b, h0:h0 + 2].reshape(2, NCHUNK, 128, S).copy()
        in_maps.append(m)

    res = run_bass_kernel_spmd(nc, in_maps, core_ids=list(range(8)),
                               trace=bool(_cache.get('trace')))
    _cache['last_result'] = res

    out = np.zeros((B, H, S, D), np.float32)
    Ab = np.zeros((B, S, S), np.float64)
    for core in range(8):
        b = core // 4
        h0 = 2 * (core % 4)
        r = res.results[core]
        out[b, h0:h0 + 2] = r['outh'].reshape(2, S, D)
        Ab[b] += r['Ap'].reshape(S, S).astype(np.float64)

    eA = np.exp(Ab)
    a = (eA / eA.sum(-1, keepdims=True)).mean(1)
    mn = a.min(-1, keepdims=True)
    mx = a.max(-1, keepdims=True)
    a = (a - mn) / (mx - mn + EPS)
    return out, a.astype(np.float32)
